# revision 17
# baseline (speedup 1.0000x reference)
"""Segment-gather-mean kernel for Trainium2 (8 NeuronCores).

out[a] = mean over edges e with ancestors[e] == a of features[curr_nodes_idx[e]]

Strategy: shard the 50000 output segments across 8 cores (6250 each). The host
buckets each core's edges by 128-segment window. Per window, edges are split by
node index (< / >= 32768, the int16 dma_gather limit) and laid out into
128-edge chunks whose size is the cross-core maximum for that window (rounded
up to full chunks) — the NEFF is shared by all 8 cores, so the schedule is
static per window but identical across cores; shorter cores pad with node 0 /
segment -1 slots that are gathered but contribute nothing.

On device, each window runs: dma_gather of its chunks (one row per partition,
4 SWDGE queues round-robin), one wide vector-engine op building all chunk
one-hot matrices (is_equal against an iota row, -1 pads give zero columns), a
PE matmul per chunk accumulating onehot.T @ rows into the window's PSUM tile,
and a scalar-engine PSUM->SBUF copy scaled by the host-precomputed 1/count,
DMA'd to the output rows.
"""

import math
import os
import sys

sys.path.insert(0, "/opt/trn_rl_repo")

import numpy as np

P = 128
D = 128
N_OUT = 50000
N_NODES = 50000
N_CORES = 8
NSEG = N_OUT // N_CORES          # segments per core
W = math.ceil(NSEG / P)          # seg windows per core
LAST_ROWS = NSEG - (W - 1) * P   # rows in the final window
SPLIT = 32768                    # int16 gather index limit

# set KERNEL_TRACE=1 to capture an NTFF profile; the BassKernelResults of the
# last run (with .exec_time_ns) is stored in `last_results`.
TRACE = os.environ.get("KERNEL_TRACE", "0") == "1"
last_results = None

_nc_cache = {}


def _build_nc(mA, mB, t_iters=1):
    """mA/mB: per-window slot counts (multiples of 128), shared by all cores."""
    import concourse.bacc as bacc
    import concourse.mybir as mybir
    from concourse.tile import TileContext

    cA = [m // P for m in mA]
    cB = [m // P for m in mB]
    cT = [a + b for a, b in zip(cA, cB)]
    Cmax = max(cT)
    n_idxa = sum(mA) // 16
    n_idxb = sum(mB) // 16
    n_seg = sum(cT)

    nc = bacc.Bacc("TRN2", target_bir_lowering=False, debug=False,
                   num_devices=N_CORES, num_swdge_queues=4)
    feat = nc.dram_tensor("feat", [N_NODES, D], mybir.dt.float32,
                          kind="ExternalInput")
    idxa = nc.dram_tensor("idxa", [P, n_idxa], mybir.dt.int16,
                          kind="ExternalInput")
    idxb = nc.dram_tensor("idxb", [P, n_idxb], mybir.dt.int16,
                          kind="ExternalInput")
    seg = nc.dram_tensor("seg", [P, n_seg], mybir.dt.float32,
                         kind="ExternalInput")
    recip = nc.dram_tensor("recip", [P, W], mybir.dt.float32,
                           kind="ExternalInput")
    iota = nc.dram_tensor("iota", [P, Cmax, P], mybir.dt.float32,
                          kind="ExternalInput")
    out = nc.dram_tensor("out", [NSEG, D], mybir.dt.float32,
                         kind="ExternalOutput")

    with TileContext(nc) as tc:
        with (
            tc.tile_pool(name="const", bufs=1) as cpool,
            tc.tile_pool(name="gath", bufs=12) as gpool,
            tc.tile_pool(name="oh", bufs=3) as ohpool,
            tc.tile_pool(name="psum", bufs=4, space="PSUM") as ppool,
            tc.tile_pool(name="osb", bufs=4) as opool,
        ):
            idxa_sb = cpool.tile([P, n_idxa], mybir.dt.int16)
            idxb_sb = cpool.tile([P, n_idxb], mybir.dt.int16)
            seg_sb = cpool.tile([P, n_seg], mybir.dt.float32)
            recip_sb = cpool.tile([P, W], mybir.dt.float32)
            iota_sb = cpool.tile([P, Cmax, P], mybir.dt.float32)
            nc.sync.dma_start(idxa_sb[:], idxa[:])
            nc.sync.dma_start(idxb_sb[:], idxb[:])
            nc.sync.dma_start(seg_sb[:], seg[:])
            nc.sync.dma_start(recip_sb[:], recip[:])
            nc.sync.dma_start(iota_sb[:], iota[:])

            def body(_=None):
                offa = 0
                offb = 0
                offc = 0
                for w in range(W):
                    ca, cb, c = cA[w], cB[w], cT[w]
                    g = gpool.tile([P, c, D], mybir.dt.float32)
                    nc.gpsimd.dma_gather(
                        g[:, :ca, :], feat[:SPLIT, :],
                        idxa_sb[:, offa: offa + mA[w] // 16],
                        mA[w], mA[w], D, single_packet=False,
                        queue_num=w % 4)
                    nc.gpsimd.dma_gather(
                        g[:, ca:, :], feat[SPLIT:, :],
                        idxb_sb[:, offb: offb + mB[w] // 16],
                        mB[w], mB[w], D, single_packet=False,
                        queue_num=(w + 2) % 4)
                    oh = ohpool.tile([P, c, P], mybir.dt.float32)
                    nc.vector.tensor_tensor(
                        out=oh[:],
                        in0=iota_sb[:, :c, :],
                        in1=seg_sb[:, offc: offc + c].to_broadcast([P, c, P]),
                        op=mybir.AluOpType.is_equal,
                    )
                    ps = ppool.tile([P, D], mybir.dt.float32, space="PSUM")
                    for j in range(c):
                        nc.tensor.matmul(
                            ps[:], lhsT=oh[:, j, :], rhs=g[:, j, :],
                            start=(j == 0), stop=(j == c - 1))
                    osb = opool.tile([P, D], mybir.dt.float32)
                    nc.scalar.activation(
                        osb[:], ps[:], mybir.ActivationFunctionType.Copy,
                        scale=recip_sb[:, w:w + 1])
                    rows = P if w < W - 1 else LAST_ROWS
                    nc.sync.dma_start(out[w * P: w * P + rows, :],
                                      osb[:rows, :])
                    offa += mA[w] // 16
                    offb += mB[w] // 16
                    offc += c

            if t_iters == 1:
                body()
            else:
                with tc.For_i(0, t_iters, 1) as _i:
                    body()

    nc.compile()
    return nc


def _prep_core(anc_l, nodes_l, mA, mB):
    """Build idxa/idxb/seg host arrays for one core given per-window sizes."""
    w_all = anc_l // P
    s_all = (anc_l % P).astype(np.float32)
    low = nodes_l < SPLIT

    totA = sum(mA)
    totB = sum(mB)
    cT = [(a + b) // P for a, b in zip(mA, mB)]
    idxA = np.zeros(totA, np.int16)
    segA = [None] * W
    idxB = np.zeros(totB, np.int16)
    segB = [None] * W

    for sel, idx_flat, seg_list, m_list, off0 in (
        (low, idxA, segA, mA, 0),
        (~low, idxB, segB, mB, SPLIT),
    ):
        wsel = w_all[sel]
        nsel = nodes_l[sel] - off0
        ssel = s_all[sel]
        order = np.argsort(wsel, kind="stable")
        wsel = wsel[order]
        nsel = nsel[order]
        ssel = ssel[order]
        cnt = np.bincount(wsel, minlength=W)
        offs = np.concatenate([[0], np.cumsum(cnt)])
        pos = 0
        for w in range(W):
            k = int(cnt[w])
            m = m_list[w]
            assert k <= m, (w, k, m)
            idx_flat[pos: pos + k] = nsel[offs[w]: offs[w] + k].astype(np.int16)
            s = np.full(m, -1.0, np.float32)
            s[:k] = ssel[offs[w]: offs[w] + k]
            seg_list[w] = s
            pos += m

    def wrap_idx(flat, m_list):
        # per window: element i -> [i % 16, i // 16]; windows along columns
        cols = []
        pos = 0
        for m in m_list:
            cols.append(flat[pos: pos + m].reshape(m // 16, 16).T)
            pos += m
        return np.tile(np.concatenate(cols, axis=1), (8, 1))

    # seg layout [128, sum(cT)]: per window, cA chunks then cB chunks;
    # column = chunk, partition = lane within chunk
    seg_cols = []
    for w in range(W):
        both = np.concatenate([segA[w], segB[w]])
        seg_cols.append(both.reshape(cT[w], P).T)
    seg_sb = np.ascontiguousarray(np.concatenate(seg_cols, axis=1))
    return wrap_idx(idxA, mA), wrap_idx(idxB, mB), seg_sb


def _prepare(features, nodes, anc):
    """Host-side sharding: returns (mA, mB, in_maps)."""
    core = anc // NSEG
    anc_local = anc - core * NSEG

    per_core = []
    cntA = np.zeros((N_CORES, W), np.int64)
    cntB = np.zeros((N_CORES, W), np.int64)
    for c in range(N_CORES):
        m = core == c
        a_l = anc_local[m]
        n_l = nodes[m]
        per_core.append((a_l, n_l))
        w_l = a_l // P
        low = n_l < SPLIT
        cntA[c] = np.bincount(w_l[low], minlength=W)
        cntB[c] = np.bincount(w_l[~low], minlength=W)

    # per-window slot counts: cross-core max, rounded up to full 128-chunks
    mA = [int(max(1, math.ceil(cntA[:, w].max() / P))) * P for w in range(W)]
    mB = [int(max(1, math.ceil(cntB[:, w].max() / P))) * P for w in range(W)]

    cnt = np.bincount(anc, minlength=N_OUT).astype(np.float32)
    recip_all = (1.0 / np.maximum(cnt, 1.0)).astype(np.float32)

    Cmax = max((a + b) // P for a, b in zip(mA, mB))
    iota = np.ascontiguousarray(
        np.tile(np.arange(P, dtype=np.float32)[None, None, :], (P, Cmax, 1)))

    in_maps = []
    for c in range(N_CORES):
        a_l, n_l = per_core[c]
        ia, ib, sg = _prep_core(a_l, n_l, mA, mB)
        r = recip_all[c * NSEG:(c + 1) * NSEG]
        r = np.concatenate([r, np.ones(W * P - NSEG, np.float32)])
        r_sb = np.ascontiguousarray(r.reshape(W, P).T)
        in_maps.append({
            "feat": features,
            "idxa": ia,
            "idxb": ib,
            "seg": sg,
            "recip": r_sb,
            "iota": iota,
        })
    return mA, mB, in_maps


def kernel(**inputs):
    from concourse.bass_utils import run_bass_kernel_spmd

    features = np.ascontiguousarray(
        np.asarray(inputs["features"], dtype=np.float32))
    nodes = np.asarray(inputs["curr_nodes_idx"]).astype(np.int64)
    anc = np.asarray(inputs["ancestors"]).astype(np.int64)
    uall = np.asarray(inputs["uall_ancestors_idx"]).astype(np.int64)

    mA, mB, in_maps = _prepare(features, nodes, anc)

    key = (tuple(mA), tuple(mB))
    if key not in _nc_cache:
        _nc_cache[key] = _build_nc(mA, mB)
    nc = _nc_cache[key]

    res = run_bass_kernel_spmd(nc, in_maps, core_ids=list(range(N_CORES)),
                               trace=TRACE)
    global last_results
    last_results = res
    mean = np.concatenate([res.results[c]["out"] for c in range(N_CORES)],
                          axis=0)
    out = np.zeros((N_OUT, D), np.float32)
    out[uall] = mean
    return out


# revision 19
# speedup vs baseline: 1.1110x; 1.1110x over previous
"""Segment-gather-mean kernel for Trainium2 (8 NeuronCores).

out[a] = mean over edges e with ancestors[e] == a of features[curr_nodes_idx[e]]

Strategy: shard the 50000 output segments across 8 cores (6250 each). The host
buckets each core's edges by 128-segment window. Per window, edges are split by
node index (< / >= 32768, the int16 dma_gather limit) and laid out into
128-edge chunks whose size is the cross-core maximum for that window (rounded
up to full chunks) — the NEFF is shared by all 8 cores, so the schedule is
static per window but identical across cores; shorter cores pad with node 0 /
segment -1 slots that are gathered but contribute nothing.

On device, each window runs: dma_gather of its chunks (one row per partition,
4 SWDGE queues round-robin), one wide vector-engine op building all chunk
one-hot matrices (is_equal against an iota row, -1 pads give zero columns), a
PE matmul per chunk accumulating onehot.T @ rows into the window's PSUM tile,
and a scalar-engine PSUM->SBUF copy scaled by the host-precomputed 1/count,
DMA'd to the output rows.
"""

import math
import os
import sys

sys.path.insert(0, "/opt/trn_rl_repo")

import numpy as np

P = 128
D = 128
N_OUT = 50000
N_NODES = 50000
N_CORES = 8
NSEG = N_OUT // N_CORES          # segments per core
W = math.ceil(NSEG / P)          # seg windows per core
LAST_ROWS = NSEG - (W - 1) * P   # rows in the final window
SPLIT = 32768                    # int16 gather index limit

# set KERNEL_TRACE=1 to capture an NTFF profile; the BassKernelResults of the
# last run (with .exec_time_ns) is stored in `last_results`.
TRACE = os.environ.get("KERNEL_TRACE", "0") == "1"
last_results = None

_nc_cache = {}


def _build_nc(mA, mB, t_iters=1):
    """mA/mB: per-window slot counts (multiples of 128), shared by all cores."""
    import concourse.bacc as bacc
    import concourse.mybir as mybir
    from concourse.tile import TileContext

    cA = [m // P for m in mA]
    cB = [m // P for m in mB]
    cT = [a + b for a, b in zip(cA, cB)]
    Cmax = max(cT)
    n_idxa = sum(mA) // 16
    n_idxb = sum(mB) // 16
    n_seg = sum(cT)

    nc = bacc.Bacc("TRN2", target_bir_lowering=False, debug=False,
                   num_devices=N_CORES, num_swdge_queues=4)
    feat = nc.dram_tensor("feat", [N_NODES, D], mybir.dt.float32,
                          kind="ExternalInput")
    idxa = nc.dram_tensor("idxa", [P, n_idxa], mybir.dt.int16,
                          kind="ExternalInput")
    idxb = nc.dram_tensor("idxb", [P, n_idxb], mybir.dt.int16,
                          kind="ExternalInput")
    seg = nc.dram_tensor("seg", [P, n_seg], mybir.dt.float32,
                         kind="ExternalInput")
    recip = nc.dram_tensor("recip", [P, W], mybir.dt.float32,
                           kind="ExternalInput")
    iota = nc.dram_tensor("iota", [P, Cmax, P], mybir.dt.float32,
                          kind="ExternalInput")
    out = nc.dram_tensor("out", [NSEG, D], mybir.dt.float32,
                         kind="ExternalOutput")

    # keep the gather pool inside ~96KB/partition whatever the distribution
    gath_bufs = min(10, max(2, (96 * 1024) // (Cmax * 512)))

    with TileContext(nc) as tc:
        with (
            tc.tile_pool(name="const", bufs=1) as cpool,
            tc.tile_pool(name="gath", bufs=gath_bufs) as gpool,
            tc.tile_pool(name="oh", bufs=3) as ohpool,
            tc.tile_pool(name="psum", bufs=4, space="PSUM") as ppool,
            tc.tile_pool(name="osb", bufs=4) as opool,
        ):
            idxa_sb = cpool.tile([P, n_idxa], mybir.dt.int16)
            idxb_sb = cpool.tile([P, n_idxb], mybir.dt.int16)
            seg_sb = cpool.tile([P, n_seg], mybir.dt.float32)
            recip_sb = cpool.tile([P, W], mybir.dt.float32)
            iota_sb = cpool.tile([P, Cmax, P], mybir.dt.float32)
            nc.sync.dma_start(idxa_sb[:], idxa[:])
            nc.sync.dma_start(idxb_sb[:], idxb[:])
            nc.sync.dma_start(seg_sb[:], seg[:])
            nc.sync.dma_start(recip_sb[:], recip[:])
            nc.sync.dma_start(iota_sb[:], iota[:])

            def body(_=None):
                offa = 0
                offb = 0
                offc = 0
                for w in range(W):
                    ca, cb, c = cA[w], cB[w], cT[w]
                    g = gpool.tile([P, c, D], mybir.dt.float32)
                    nc.gpsimd.dma_gather(
                        g[:, :ca, :], feat[:SPLIT, :],
                        idxa_sb[:, offa: offa + mA[w] // 16],
                        mA[w], mA[w], D, single_packet=False,
                        queue_num=w % 4)
                    nc.gpsimd.dma_gather(
                        g[:, ca:, :], feat[SPLIT:, :],
                        idxb_sb[:, offb: offb + mB[w] // 16],
                        mB[w], mB[w], D, single_packet=False,
                        queue_num=(w + 2) % 4)
                    oh = ohpool.tile([P, c, P], mybir.dt.float32)
                    nc.vector.tensor_tensor(
                        out=oh[:],
                        in0=iota_sb[:, :c, :],
                        in1=seg_sb[:, offc: offc + c].to_broadcast([P, c, P]),
                        op=mybir.AluOpType.is_equal,
                    )
                    ps = ppool.tile([P, D], mybir.dt.float32, space="PSUM")
                    for j in range(c):
                        nc.tensor.matmul(
                            ps[:], lhsT=oh[:, j, :], rhs=g[:, j, :],
                            start=(j == 0), stop=(j == c - 1))
                    osb = opool.tile([P, D], mybir.dt.float32)
                    nc.scalar.activation(
                        osb[:], ps[:], mybir.ActivationFunctionType.Copy,
                        scale=recip_sb[:, w:w + 1])
                    rows = P if w < W - 1 else LAST_ROWS
                    nc.sync.dma_start(out[w * P: w * P + rows, :],
                                      osb[:rows, :])
                    offa += mA[w] // 16
                    offb += mB[w] // 16
                    offc += c

            if t_iters == 1:
                body()
            else:
                with tc.For_i(0, t_iters, 1) as _i:
                    body()

    nc.compile()
    return nc


def _prep_core(anc_l, nodes_l, mA, mB):
    """Build idxa/idxb/seg host arrays for one core given per-window sizes."""
    w_all = anc_l // P
    s_all = (anc_l % P).astype(np.float32)
    low = nodes_l < SPLIT

    totA = sum(mA)
    totB = sum(mB)
    cT = [(a + b) // P for a, b in zip(mA, mB)]
    idxA = np.zeros(totA, np.int16)
    segA = [None] * W
    idxB = np.zeros(totB, np.int16)
    segB = [None] * W

    for sel, idx_flat, seg_list, m_list, off0 in (
        (low, idxA, segA, mA, 0),
        (~low, idxB, segB, mB, SPLIT),
    ):
        wsel = w_all[sel]
        nsel = nodes_l[sel] - off0
        ssel = s_all[sel]
        order = np.argsort(wsel, kind="stable")
        wsel = wsel[order]
        nsel = nsel[order]
        ssel = ssel[order]
        cnt = np.bincount(wsel, minlength=W)
        offs = np.concatenate([[0], np.cumsum(cnt)])
        pos = 0
        for w in range(W):
            k = int(cnt[w])
            m = m_list[w]
            assert k <= m, (w, k, m)
            idx_flat[pos: pos + k] = nsel[offs[w]: offs[w] + k].astype(np.int16)
            s = np.full(m, -1.0, np.float32)
            s[:k] = ssel[offs[w]: offs[w] + k]
            seg_list[w] = s
            pos += m

    def wrap_idx(flat, m_list):
        # per window: element i -> [i % 16, i // 16]; windows along columns
        cols = []
        pos = 0
        for m in m_list:
            cols.append(flat[pos: pos + m].reshape(m // 16, 16).T)
            pos += m
        return np.tile(np.concatenate(cols, axis=1), (8, 1))

    # seg layout [128, sum(cT)]: per window, cA chunks then cB chunks;
    # column = chunk, partition = lane within chunk
    seg_cols = []
    for w in range(W):
        both = np.concatenate([segA[w], segB[w]])
        seg_cols.append(both.reshape(cT[w], P).T)
    seg_sb = np.ascontiguousarray(np.concatenate(seg_cols, axis=1))
    return wrap_idx(idxA, mA), wrap_idx(idxB, mB), seg_sb


def _prepare(features, nodes, anc):
    """Host-side sharding: returns (mA, mB, in_maps)."""
    core = anc // NSEG
    anc_local = anc - core * NSEG

    per_core = []
    cntA = np.zeros((N_CORES, W), np.int64)
    cntB = np.zeros((N_CORES, W), np.int64)
    for c in range(N_CORES):
        m = core == c
        a_l = anc_local[m]
        n_l = nodes[m]
        per_core.append((a_l, n_l))
        w_l = a_l // P
        low = n_l < SPLIT
        cntA[c] = np.bincount(w_l[low], minlength=W)
        cntB[c] = np.bincount(w_l[~low], minlength=W)

    # per-window slot counts: cross-core max, rounded up to full 128-chunks
    mA = [int(max(1, math.ceil(cntA[:, w].max() / P))) * P for w in range(W)]
    mB = [int(max(1, math.ceil(cntB[:, w].max() / P))) * P for w in range(W)]

    cnt = np.bincount(anc, minlength=N_OUT).astype(np.float32)
    recip_all = (1.0 / np.maximum(cnt, 1.0)).astype(np.float32)

    Cmax = max((a + b) // P for a, b in zip(mA, mB))
    iota = np.ascontiguousarray(
        np.tile(np.arange(P, dtype=np.float32)[None, None, :], (P, Cmax, 1)))

    in_maps = []
    for c in range(N_CORES):
        a_l, n_l = per_core[c]
        ia, ib, sg = _prep_core(a_l, n_l, mA, mB)
        r = recip_all[c * NSEG:(c + 1) * NSEG]
        r = np.concatenate([r, np.ones(W * P - NSEG, np.float32)])
        r_sb = np.ascontiguousarray(r.reshape(W, P).T)
        in_maps.append({
            "feat": features,
            "idxa": ia,
            "idxb": ib,
            "seg": sg,
            "recip": r_sb,
            "iota": iota,
        })
    return mA, mB, in_maps


def kernel(**inputs):
    from concourse.bass_utils import run_bass_kernel_spmd

    features = np.ascontiguousarray(
        np.asarray(inputs["features"], dtype=np.float32))
    nodes = np.asarray(inputs["curr_nodes_idx"]).astype(np.int64)
    anc = np.asarray(inputs["ancestors"]).astype(np.int64)
    uall = np.asarray(inputs["uall_ancestors_idx"]).astype(np.int64)

    mA, mB, in_maps = _prepare(features, nodes, anc)

    key = (tuple(mA), tuple(mB))
    if key not in _nc_cache:
        _nc_cache[key] = _build_nc(mA, mB)
    nc = _nc_cache[key]

    res = run_bass_kernel_spmd(nc, in_maps, core_ids=list(range(N_CORES)),
                               trace=TRACE)
    global last_results
    last_results = res
    mean = np.concatenate([res.results[c]["out"] for c in range(N_CORES)],
                          axis=0)
    out = np.zeros((N_OUT, D), np.float32)
    out[uall] = mean
    return out
